# revision 12
# baseline (speedup 1.0000x reference)
"""Trainium2 Bass kernel for batched 3x3 VALID conv (NCHW / OIHW).

x: [32, 128, 64, 64] f32, weight: [256, 128, 3, 3] f32 -> out: [32, 256, 62, 62] f32.

Strategy: data-parallel over batch across 8 NeuronCores (4 images each),
with 1D Winograd F(2,3) along H to cut PE work by 1/3 vs the direct
9-tap shift-matmul (12 taps at half the free dim instead of 9 at full).

Per output-row pair (2t, 2t+1) the H-axis is factored as
  M_u[co, t, w] = sum_dx  W'[u,dx][ci,co].T @ Xt_u[ci, t, dx+w]
  out[2t]   = M0 + M1 + M2
  out[2t+1] = M1 - M2 - M3
where Xt_u are the B^T-transformed input planes (computed on host along
with the G-transformed weights, both shipped as fp16) and the 3-term
inverse combines on-device: ScalarE evacuates M1/M2/M3 from PSUM to
SBUF fp16; VectorE does the 4 add/subs (2 of them at 2x fp16 rate).

PSUM: per t-group of 8 Winograd tiles, 4 banks (M0..M3) of [co, 8, 62]
fp32; two t-groups rotate through the 8 banks so accumulation of group
g+1 overlaps the drain of group g. Everything is fp16 on the PE
(same rate as bf16, 4 extra mantissa bits): end-to-end rel err ~5e-4.
"""

import numpy as np

_B, _CIN, _H, _W = 32, 128, 64, 64
_COUT = 256
_HO, _WO = 62, 62
_NCORES = 8
_BPC = _B // _NCORES  # images per core
_NT = 31  # Winograd output-row-pair tiles
_NU = 4  # Winograd taps along H

# u-slots are stored in compute order (M1, M2, M0, M3) so weight/plane
# chunks stream in exactly the order the PE consumes them.
_U_ORDER = [1, 2, 0, 3]
_S_M1, _S_M2, _S_M0, _S_M3 = 0, 1, 2, 3

_TGROUPS = [(0, 8), (8, 8), (16, 8), (24, 7)]

_nc_cache = None


def _build():
    global _nc_cache
    if _nc_cache is not None:
        return _nc_cache

    import concourse.bass as bass
    import concourse.mybir as mybir
    from concourse import bacc
    from concourse.tile import TileContext

    f32 = mybir.dt.float32
    f16 = mybir.dt.float16

    nc = bacc.Bacc("TRN2", target_bir_lowering=False)
    xt_d = nc.dram_tensor("xt", [_BPC, _CIN, _NU, _NT, _W], f16, kind="ExternalInput")
    w_d = nc.dram_tensor("w", [_CIN, 2, 3 * _NU, 128], f16, kind="ExternalInput")
    # Output in (parity, t) plane-major layout: o[img, co, p, t, w] holds
    # out[img, co, 2t+p, w]. Keeps every store fully contiguous per
    # partition (992B runs instead of 124B); host interleaves the rows.
    o_d = nc.dram_tensor("o", [_BPC, _COUT, 2, _NT, _WO], f16, kind="ExternalOutput")

    with TileContext(nc) as tc:
        with (
            tc.tile_pool(name="wpool", bufs=1) as wpool,
            tc.tile_pool(name="xpool", bufs=2) as xpool,
            tc.tile_pool(name="mpool", bufs=9) as mpool,
            tc.tile_pool(name="tupool", bufs=6) as tupool,
            tc.tile_pool(name="opool", bufs=4) as opool,
            tc.tile_pool(name="pspool", bufs=8, space=bass.MemorySpace.PSUM) as pspool,
        ):
            w_sb = wpool.tile([_CIN, 2, 3 * _NU, 128], f16)
            x_tile_a = xpool.tile([_CIN, _NU, _NT, _W], f16, tag="x")
            x_tile_b = xpool.tile([_CIN, _NU, _NT, _W], f16, tag="x")
            x_tiles = [x_tile_a, x_tile_b]

            # PE warmup on a zeroed fp16 tile: keeps the PE continuously
            # busy from the end of the prologue so the HAM clock (which
            # also gates DMA throughput) ramps to full speed before the
            # real matmuls and bulk DMA traffic need it.
            wup = wpool.tile([128, 512], f16)
            wps = pspool.tile([128, 512], f32, tag="ps")
            dummy = wpool.tile([128, 512], f16)
            nc.vector.memset(wup[:], 0)
            for _ in range(12):
                nc.vector.tensor_copy(dummy[:], wup[:])
            for _ in range(5):
                nc.tensor.matmul(wps[:], wup[:, 0:128], wup[:], start=True, stop=True)
            for _ in range(8):
                nc.tensor.matmul(
                    wps[:, 0:128], wup[:, 0:128], wup[:, 0:128], start=True, stop=True
                )

            # Head DMAs, sequenced in exact consumption order so the first
            # t-groups' data never queues behind lower-priority traffic.
            # Group g needs all 4 slot-chunks [8g:8g+8] plus the weights;
            # the three rings each carry a consumption-ordered share.
            # img1's prefetch is deferred into the loop for the same reason.
            # The 393KB ct0-weight block is the longest critical item, so
            # it is split across both HWDGE rings (first 6 taps on sync,
            # last 6 on scalar) instead of serializing on one.
            xs0 = x_tiles[0]
            nc.sync.dma_start(w_sb[:, 0, 0:6, :], w_d[:, 0, 0:6, :])
            nc.scalar.dma_start(w_sb[:, 0, 6:12, :], w_d[:, 0, 6:12, :])
            nc.gpsimd.dma_start(xs0[:, 2, 0:8, :], xt_d[0, :, 2, 0:8, :])
            nc.sync.dma_start(xs0[:, 0, 0:8, :], xt_d[0, :, 0, 0:8, :])
            nc.scalar.dma_start(xs0[:, 1, 0:8, :], xt_d[0, :, 1, 0:8, :])
            nc.gpsimd.dma_start(xs0[:, 3, 0:8, :], xt_d[0, :, 3, 0:8, :])
            nc.sync.dma_start(xs0[:, 2, 8:16, :], xt_d[0, :, 2, 8:16, :])
            nc.scalar.dma_start(xs0[:, 0, 8:16, :], xt_d[0, :, 0, 8:16, :])
            nc.gpsimd.dma_start(xs0[:, 1, 8:16, :], xt_d[0, :, 1, 8:16, :])
            nc.sync.dma_start(xs0[:, 3, 8:16, :], xt_d[0, :, 3, 8:16, :])
            nc.scalar.dma_start(xs0[:, 1, 16:31, :], xt_d[0, :, 1, 16:31, :])
            nc.gpsimd.dma_start(xs0[:, 0, 16:31, :], xt_d[0, :, 0, 16:31, :])
            nc.scalar.dma_start(xs0[:, 2, 16:31, :], xt_d[0, :, 2, 16:31, :])
            nc.gpsimd.dma_start(xs0[:, 3, 16:31, :], xt_d[0, :, 3, 16:31, :])
            nc.sync.dma_start(w_sb[:, 1], w_d[:, 1])
            store_ctr = [0]

            def mm(ps, x_sb, ct, slot, tg0, ntg, dx, start, stop):
                nc.tensor.matmul(
                    ps[:, 0:ntg, :],
                    w_sb[:, ct, slot * 3 + dx, :],
                    x_sb[:, slot, tg0 : tg0 + ntg, dx : dx + _WO],
                    start=start,
                    stop=stop,
                )

            for img in range(_BPC):
                x_sb = x_tiles[img % 2]
                for ct in range(_COUT // 128):
                    groups = _TGROUPS
                    if img == _BPC - 1 and ct == 1:
                        # Split the final t-group so the end-of-kernel drain
                        # (copy + combine + store + completion) covers 3
                        # tiles instead of 7.
                        groups = [(0, 8), (8, 8), (16, 8), (24, 5), (29, 2)]
                    for gi, (tg0, ntg) in enumerate(groups):
                        # Prefetch image img+1 after the first group's MMs
                        # are issued (img0's own chunks have priority at the
                        # head; steady-state has ~20us of slack per image).
                        if gi == 3 and ct == 0 and img + 1 < _BPC:
                            nc.scalar.dma_start(
                                x_tiles[(img + 1) % 2][:], xt_d[img + 1]
                            )
                        ps_l = [
                            pspool.tile([128, ntg, _WO], f32, tag="ps", name="ps")
                            for _ in range(_NU)
                        ]
                        for slot in range(_NU):
                            for dx in range(3):
                                mm(ps_l[slot], x_sb, ct, slot, tg0, ntg, dx,
                                   start=(dx == 0), stop=(dx == 2))
                        # Drain: ScalarE evacuates M1/M2 to fp16 SBUF,
                        # VectorE combines (Ye at 2x fp16 rate; the two ops
                        # with a PSUM operand run at 1x). Ye/Yo land in
                        # separate contiguous planes; the store DMA
                        # interleaves the row pairs on the way to HBM.
                        m1 = mpool.tile([128, ntg, _WO], f16, tag="m")
                        m2 = mpool.tile([128, ntg, _WO], f16, tag="m")
                        nc.scalar.copy(m1[:], ps_l[_S_M1][:, 0:ntg, :])
                        nc.scalar.copy(m2[:], ps_l[_S_M2][:, 0:ntg, :])
                        uu = tupool.tile([128, ntg, _WO], f16, tag="tu")
                        tt = tupool.tile([128, ntg, _WO], f16, tag="tu")
                        # uu is pure-SBUF fp16 work: GPSIMD handles it (idle
                        # otherwise), freeing DVE for the PSUM-reading ops.
                        # End-game exception: gpsimd's SWDGE queue drain
                        # (~3us after its last store) must not gate the
                        # final drains, so the last ct keeps uu on DVE.
                        u_eng = (
                            nc.vector
                            if img == _BPC - 1 and ct == 1
                            else nc.gpsimd
                        )
                        # tt is issued first: it only needs m1 (ScalarE's
                        # first copy), so when uu shares the DVE queue in
                        # the end-game, DVE starts ~250ns sooner.
                        nc.vector.tensor_add(tt[:], ps_l[_S_M0][:, 0:ntg, :], m1[:])
                        u_eng.tensor_sub(uu[:], m1[:], m2[:])
                        ot = opool.tile([128, 2, ntg, _WO], f16, tag="st")
                        nc.vector.tensor_add(ot[:, 0, :, :], tt[:], m2[:])
                        nc.vector.tensor_sub(
                            ot[:, 1, :, :], uu[:], ps_l[_S_M3][:, 0:ntg, :]
                        )
                        o_slice = o_d[
                            img, ct * 128 : (ct + 1) * 128, :, tg0 : tg0 + ntg, :
                        ]
                        # No gpsimd stores for the last image: its SWDGE
                        # drain would otherwise land inside the kernel tail.
                        late = img == _BPC - 1 and ct == 1 and gi >= len(groups) - 3
                        if late:
                            q = nc.sync
                        elif img == _BPC - 1:
                            q = nc.sync if store_ctr[0] % 2 == 0 else nc.scalar
                        else:
                            q = nc.sync if store_ctr[0] % 2 == 0 else nc.gpsimd
                        store_ctr[0] += 1
                        q.dma_start(o_slice, ot[:])

    nc.compile()
    _nc_cache = nc
    return nc


def _prep_in_maps(x, weight):
    x = np.asarray(x, dtype=np.float32)
    w = np.asarray(weight, dtype=np.float32)
    assert x.shape == (_B, _CIN, _H, _W), x.shape
    assert w.shape == (_COUT, _CIN, 3, 3), w.shape

    # Weight transform: W'[u,dx][ci,co] = sum_dy G[u,dy] w[co,ci,dy,dx],
    # stored tap-major in compute order: [ci, uo*3+dx, co].
    G = np.array(
        [[1, 0, 0], [0.5, 0.5, 0.5], [0.5, -0.5, 0.5], [0, 0, 1]], dtype=np.float32
    )
    wt = np.einsum("uy,ocyx->uxco", G, w)[_U_ORDER]  # [4, 3, ci, co]
    wt = wt.reshape(3 * _NU, _CIN, 2, 128).transpose(1, 2, 0, 3)  # [ci, 2ct, 12, 128]
    wt = np.ascontiguousarray(wt).astype(np.float16)

    # Input transform along H (B^T): planes for t = 0..30 from rows
    # {2t, 2t+1, 2t+2, 2t+3}, in compute order (M1, M2, M0, M3).
    xs = x.reshape(_NCORES, _BPC, _CIN, _H, _W)
    d0 = xs[:, :, :, 0:61:2, :]
    d1 = xs[:, :, :, 1:62:2, :]
    d2 = xs[:, :, :, 2:63:2, :]
    d3 = xs[:, :, :, 3:64:2, :]
    xt = np.stack([d0 - d2, d1 + d2, d2 - d1, d1 - d3], axis=3)  # [8,4,ci,u,31,64]
    xt = xt[:, :, :, _U_ORDER].astype(np.float16)
    return [
        {"xt": np.ascontiguousarray(xt[i]), "w": wt} for i in range(_NCORES)
    ]


def _run(x, weight, **kwargs):
    from concourse.bass_utils import run_bass_kernel_spmd

    nc = _build()
    res = run_bass_kernel_spmd(
        nc, _prep_in_maps(x, weight), core_ids=list(range(_NCORES)), **kwargs
    )
    # o is [BPC, COUT, 2, 31, 62] with out rows 2t+p at [.., p, t, ..].
    out = np.concatenate([r["o"] for r in res.results], axis=0)
    out = out.transpose(0, 1, 3, 2, 4).reshape(_B, _COUT, _HO, _WO)
    return out.astype(np.float32), res


def kernel(x, weight):
    out, _ = _run(x, weight)
    return out
